# revision 2
# baseline (speedup 1.0000x reference)
"""Trainium2 kernel for nn_MinNormSolverFW: min-norm Frank-Wolfe over 8 task
gradients of dimension 16777216.

Strategy (matches the sharding hint): shard the d axis across the 8 cores.
Each core computes a partial Gram matrix of its shard on the tensor engine;
the host sums the tiny partial Grams and runs the (negligible) Frank-Wolfe
loop, replicating the reference's fp32 semantics.

Device compute layout: the host pre-packs each core's shard so that every
128-column SBUF group holds 16 d-chunks x 8 vectors (columns m = i*16 + cc,
partitions = 128 d's per chunk, 2 contraction planes per column via
DoubleRow).  A single self-matmul (lhsT = rhs = group) accumulates all 16
chunk-level 8x8 outer products at full PE width into one [128,128] PSUM
region.  The host extracts the 16 diagonal 8x8 blocks of the output.

Input is quantized to fp8-e4m3 on the host (the final FW solution moves by
~4e-5 relative, far below tolerance), which quarters HBM traffic vs fp32.

v2 structure: every streaming tile gets its own dedicated SBUF buffer (the
full 16 MiB shard fits in SBUF), so DMAs never wait on buffer recycling and
stream back-to-back at fabric rate; the PE warm-up is only long enough to
cover the first tile's DMA flight; tiles ramp small -> large -> small so
the PE starts early and finishes right behind the last DMA.
"""
import numpy as np

N = 8                     # number of task vectors
D = 16777216              # vector dimension
NCORES = 8
CC = 16                   # d-chunks packed per matmul group (CC * N = 128)
DC = D // NCORES          # d per core
TOTAL_COLS = DC * N // 128 // 2  # double-row columns per core = 65536

MAX_ITER = 250
STOP_CRIT = 1e-06


def default_tile_schedule(big=4096, total=TOTAL_COLS,
                          head=(512, 1024, 2048), tail=(2048, 1024, 512)):
    head, tail = list(head), list(tail)
    mid_total = total - sum(head) - sum(tail)
    mid = [big] * (mid_total // big)
    rem = mid_total - big * len(mid)
    if rem:
        mid.append(rem)
    sched = head + mid + tail
    assert sum(sched) == total and all(c % 128 == 0 for c in sched)
    return sched


_CACHE = {}


def _np_dt(in_dt):
    if in_dt == "bfloat16":
        import ml_dtypes
        return ml_dtypes.bfloat16
    if in_dt == "float8e4":
        import ml_dtypes
        return ml_dtypes.float8_e4m3
    if in_dt == "float8e3":
        import ml_dtypes
        return ml_dtypes.float8_e3m4
    return np.float32


def _build_nc(schedule, in_dt="float8e4", warm=8, two_queues=True, swi=False):
    from concourse import bacc
    import concourse.mybir as mybir
    from concourse.tile import TileContext

    dt = getattr(mybir.dt, in_dt)
    total_cols = sum(schedule)
    n_mm = total_cols // 128
    perf_mode = (mybir.MatmulPerfMode.DoubleRowSwInterleave if swi
                 else mybir.MatmulPerfMode.DoubleRow)
    nc = bacc.Bacc("TRN2", debug=False)
    x = nc.dram_tensor("x", [128 * 2 * total_cols], dt, kind="ExternalInput")
    g_out = nc.dram_tensor("g", [1, 128, 128], mybir.dt.float32,
                           kind="ExternalOutput")
    with TileContext(nc) as tc:
        with tc.tile_pool(name="data", bufs=1) as pool, \
             tc.tile_pool(name="acc", bufs=1, space="PSUM") as ppool, \
             tc.tile_pool(name="warm", bufs=1) as wpool, \
             tc.tile_pool(name="res", bufs=1) as opool:
            acc = ppool.tile([128, 128], mybir.dt.float32)
            if warm:
                # Short PE pre-warm: overlaps the first tile's DMA flight and
                # starts the HAM activity window early.
                wt = wpool.tile([128, 128], mybir.dt.bfloat16)
                wacc = ppool.tile([128, 128], mybir.dt.float32, tag="wacc")
                nc.gpsimd.memset(wt[:], 0)
                for _ in range(warm):
                    nc.tensor.matmul(wacc[:], wt[:], wt[:],
                                     start=True, stop=True)
            k = 0
            off = 0
            for ti, cols in enumerate(schedule):
                # dedicated buffer per tile: no recycling dependencies
                if swi:
                    tile = pool.tile([128, 2 * cols], dt, tag=f"data{ti}",
                                     name=f"tile{ti}")
                else:
                    tile = pool.tile([128, 2, cols], dt, tag=f"data{ti}",
                                     name=f"tile{ti}")
                src = x[off:off + 256 * cols].rearrange(
                    "(p f) -> p f", p=128)
                if not swi:
                    src = src.rearrange("p (r c) -> p r c", r=2)
                eng = nc.scalar if (two_queues and ti % 2) else nc.sync
                eng.dma_start(out=tile[:], in_=src)
                off += 256 * cols
                for g in range(cols // 128):
                    if swi:
                        flat = tile[:, g * 256:(g + 1) * 256]
                        lhsT = flat
                        rhs = flat.rearrange("p (f two) -> p two f", two=2)
                    else:
                        lhsT = rhs = tile[:, :, g * 128:(g + 1) * 128]
                    nc.tensor.matmul(acc[:], lhsT, rhs,
                                     start=(k == 0),
                                     stop=(k == n_mm - 1),
                                     perf_mode=perf_mode)
                    k += 1
            res = opool.tile([128, 128], mybir.dt.float32, tag="res")
            nc.vector.tensor_copy(res[:], acc[:])
            nc.sync.dma_start(out=g_out[0], in_=res[:])
    assert k == n_mm
    nc.compile()
    return nc


def _pack(vecs: np.ndarray, schedule, in_dt="float8e4", swi=False,
          ncores=NCORES) -> np.ndarray:
    """[N, D] -> [ncores, 256*total_cols] flat packed device layout.

    Each 128-column matmul group holds 16 d-chunks x 8 vectors
    (column = i*16 + cc); a chunk spans 256 d's indexed by partition p
    and double-row plane r.  With swi, each group's 256 bytes per
    partition are stored interleaved-reversed: (A127,B127,...,A0,B0)
    where Ac/Bc are planes r=0/1 of logical column c, which is the
    layout DoubleRowSwInterleave's LDWEIGHTS reads contiguously.
    """
    np_dt = _np_dt(in_dt)
    dc = D // ncores
    q = vecs.astype(np_dt)
    out = np.empty((ncores, 256 * sum(schedule)), dtype=np_dt)
    for c in range(ncores):
        doff = 0
        eoff = 0
        Vc = q[:, c * dc:(c + 1) * dc]
        for cols in schedule:
            dspan = 256 * cols // N   # d per vector in this tile
            groups = cols // 128
            V = Vc[:, doff:doff + dspan].reshape(N, 128, 2, groups, CC)
            T = np.transpose(V, (1, 2, 3, 0, 4))     # [p, r, g, i, cc]
            n_el = 256 * cols
            if swi:
                # [p, r, g, col] -> [p, g, col(reversed), r]
                B = T.reshape(128, 2, groups, 128)
                W = np.transpose(B[:, :, :, ::-1], (0, 2, 3, 1))
                out[c, eoff:eoff + n_el] = W.reshape(-1)
            else:
                out[c, eoff:eoff + n_el] = T.reshape(-1)
            doff += dspan
            eoff += n_el
    return out


def _gram_from_outputs(outs, swi=False) -> np.ndarray:
    """Sum the 16 diagonal 8x8 blocks of each core's [.,128,128] output."""
    G = np.zeros((N, N), dtype=np.float64)
    for O in outs:
        O = np.asarray(O, dtype=np.float64).reshape(-1, 128, 128)
        if swi:
            O = O[:, :, ::-1]   # moving operand streamed columns reversed
        O4 = O.reshape(-1, N, CC, N, CC)
        G += np.einsum('kicjc->ij', O4)
    return G


def _fw_solve(G: np.ndarray) -> np.ndarray:
    """Frank-Wolfe min-norm loop, replicating the reference fp32 semantics."""
    G = G.astype(np.float32)
    one = np.float32(1.0)
    sol = np.full(N, 1.0 / N, dtype=np.float32)
    for _ in range(MAX_ITER):
        gram_dot_sol = G @ sol
        t = int(np.argmin(gram_dot_sol))
        v1v1 = np.float32(np.dot(sol, gram_dot_sol))
        v1v2 = np.float32(np.dot(sol, G[:, t]))
        v2v2 = G[t, t]
        denom = np.float32(v1v1 + v2v2 - np.float32(2.0) * v1v2)
        with np.errstate(divide="ignore", invalid="ignore"):
            gamma = np.float32((v2v2 - v1v2) / denom)
        if v1v2 >= v2v2:
            gamma = np.float32(0.001)
        if v1v2 >= v1v1:
            gamma = np.float32(0.999)
        new_sol = (gamma * sol).astype(np.float32)
        new_sol[t] = np.float32(new_sol[t] + (one - gamma))
        change = np.float32(np.sum(np.abs(new_sol - sol)))
        sol = new_sol
        if change < np.float32(STOP_CRIT):
            break
    return sol


CONFIG = dict(in_dt="float8e4", warm=8, two_queues=True, swi=False)
SCHEDULE = default_tile_schedule()


def kernel(vecs) -> np.ndarray:
    from concourse.bass_utils import run_bass_kernel_spmd

    vecs = np.ascontiguousarray(np.asarray(vecs, dtype=np.float32))
    assert vecs.shape == (N, D)

    X = _pack(vecs, SCHEDULE, in_dt=CONFIG["in_dt"], swi=CONFIG["swi"])
    if "nc" not in _CACHE:
        _CACHE["nc"] = _build_nc(SCHEDULE, **CONFIG)
    nc = _CACHE["nc"]
    in_maps = [{"x": X[c]} for c in range(NCORES)]
    rr = run_bass_kernel_spmd(nc, in_maps, list(range(NCORES)))
    G = _gram_from_outputs((rr.results[c]["g"] for c in range(NCORES)),
                           swi=CONFIG["swi"])
    return _fw_solve(G)


# revision 3
# speedup vs baseline: 1.0502x; 1.0502x over previous
"""Trainium2 kernel for nn_MinNormSolverFW: min-norm Frank-Wolfe over 8 task
gradients of dimension 16777216.

Strategy (matches the sharding hint): shard the d axis across the 8 cores.
Each core computes a partial Gram matrix of its shard on the tensor engine;
the host sums the tiny partial Grams and runs the (negligible) Frank-Wolfe
loop, replicating the reference's fp32 semantics.

Device compute layout: the host pre-packs each core's shard so that every
128-column SBUF group holds 16 d-chunks x 8 vectors (columns m = i*16 + cc,
partitions = 128 d's per chunk, 2 contraction planes per column via
DoubleRow).  A single self-matmul (lhsT = rhs = group) accumulates all 16
chunk-level 8x8 outer products at full PE width into one [128,128] PSUM
region.  The host extracts the 16 diagonal 8x8 blocks of the output.

Input is quantized to fp8-e4m3 on the host (the final FW solution moves by
~4e-5 relative, far below tolerance), which quarters HBM traffic vs fp32.

v2 structure: every streaming tile gets its own dedicated SBUF buffer (the
full 16 MiB shard fits in SBUF), so DMAs never wait on buffer recycling and
stream back-to-back at fabric rate; the PE warm-up is only long enough to
cover the first tile's DMA flight; tiles ramp small -> large -> small so
the PE starts early and finishes right behind the last DMA.
"""
import numpy as np

N = 8                     # number of task vectors
D = 16777216              # vector dimension
NCORES = 8
CC = 16                   # d-chunks packed per matmul group (CC * N = 128)
DC = D // NCORES          # d per core
TOTAL_COLS = DC * N // 128 // 2  # double-row columns per core = 65536

MAX_ITER = 250
STOP_CRIT = 1e-06


def default_tile_schedule(big=4096, total=TOTAL_COLS,
                          head=(512, 1024, 2048), tail=(2048, 1024, 512)):
    head, tail = list(head), list(tail)
    mid_total = total - sum(head) - sum(tail)
    mid = [big] * (mid_total // big)
    rem = mid_total - big * len(mid)
    if rem:
        mid.append(rem)
    sched = head + mid + tail
    assert sum(sched) == total and all(c % 128 == 0 for c in sched)
    return sched


_CACHE = {}


def _np_dt(in_dt):
    if in_dt == "bfloat16":
        import ml_dtypes
        return ml_dtypes.bfloat16
    if in_dt == "float8e4":
        import ml_dtypes
        return ml_dtypes.float8_e4m3
    if in_dt == "float8e3":
        import ml_dtypes
        return ml_dtypes.float8_e3m4
    return np.float32


def _build_nc(schedule, in_dt="float8e4", warm=8, two_queues=True, swi=False):
    from concourse import bacc
    import concourse.mybir as mybir
    from concourse.tile import TileContext

    dt = getattr(mybir.dt, in_dt)
    total_cols = sum(schedule)
    n_mm = total_cols // 128
    perf_mode = (mybir.MatmulPerfMode.DoubleRowSwInterleave if swi
                 else mybir.MatmulPerfMode.DoubleRow)
    nc = bacc.Bacc("TRN2", debug=False)
    x = nc.dram_tensor("x", [128 * 2 * total_cols], dt, kind="ExternalInput")
    g_out = nc.dram_tensor("g", [1, 128, 128], mybir.dt.float32,
                           kind="ExternalOutput")
    with TileContext(nc) as tc:
        with tc.tile_pool(name="data", bufs=1) as pool, \
             tc.tile_pool(name="acc", bufs=1, space="PSUM") as ppool, \
             tc.tile_pool(name="warm", bufs=1) as wpool, \
             tc.tile_pool(name="res", bufs=1) as opool:
            acc = ppool.tile([128, 128], mybir.dt.float32)
            if warm:
                # Short PE pre-warm: overlaps the first tile's DMA flight and
                # starts the HAM activity window early.
                wt = wpool.tile([128, 128], mybir.dt.bfloat16)
                wacc = ppool.tile([128, 128], mybir.dt.float32, tag="wacc")
                nc.gpsimd.memset(wt[:], 0)
                for _ in range(warm):
                    nc.tensor.matmul(wacc[:], wt[:], wt[:],
                                     start=True, stop=True)
            k = 0
            off = 0
            for ti, cols in enumerate(schedule):
                # dedicated buffer per tile: no recycling dependencies
                if swi:
                    tile = pool.tile([128, 2 * cols], dt, tag=f"data{ti}",
                                     name=f"tile{ti}")
                else:
                    tile = pool.tile([128, 2, cols], dt, tag=f"data{ti}",
                                     name=f"tile{ti}")
                src = x[off:off + 256 * cols].rearrange(
                    "(p f) -> p f", p=128)
                if not swi:
                    src = src.rearrange("p (r c) -> p r c", r=2)
                eng = nc.scalar if (two_queues and ti % 2) else nc.sync
                eng.dma_start(out=tile[:], in_=src)
                off += 256 * cols
                for g in range(cols // 128):
                    if swi:
                        flat = tile[:, g * 256:(g + 1) * 256]
                        lhsT = flat
                        rhs = flat.rearrange("p (f two) -> p two f", two=2)
                    else:
                        lhsT = rhs = tile[:, :, g * 128:(g + 1) * 128]
                    nc.tensor.matmul(acc[:], lhsT, rhs,
                                     start=(k == 0),
                                     stop=(k == n_mm - 1),
                                     perf_mode=perf_mode)
                    k += 1
            res = opool.tile([128, 128], mybir.dt.float32, tag="res")
            nc.vector.tensor_copy(res[:], acc[:])
            nc.sync.dma_start(out=g_out[0], in_=res[:])
    assert k == n_mm
    nc.compile()
    return nc


def _pack(vecs: np.ndarray, schedule, in_dt="float8e4", swi=False,
          ncores=NCORES) -> np.ndarray:
    """[N, D] -> [ncores, 256*total_cols] flat packed device layout.

    Each 128-column matmul group holds 16 d-chunks x 8 vectors
    (column = i*16 + cc); a chunk spans 256 d's indexed by partition p
    and double-row plane r.  With swi, each group's 256 bytes per
    partition are stored interleaved-reversed: (A127,B127,...,A0,B0)
    where Ac/Bc are planes r=0/1 of logical column c, which is the
    layout DoubleRowSwInterleave's LDWEIGHTS reads contiguously.
    """
    np_dt = _np_dt(in_dt)
    dc = D // ncores
    q = vecs.astype(np_dt)
    out = np.empty((ncores, 256 * sum(schedule)), dtype=np_dt)
    for c in range(ncores):
        doff = 0
        eoff = 0
        Vc = q[:, c * dc:(c + 1) * dc]
        for cols in schedule:
            dspan = 256 * cols // N   # d per vector in this tile
            groups = cols // 128
            V = Vc[:, doff:doff + dspan].reshape(N, 128, 2, groups, CC)
            T = np.transpose(V, (1, 2, 3, 0, 4))     # [p, r, g, i, cc]
            n_el = 256 * cols
            if swi:
                # [p, r, g, col] -> [p, g, col(reversed), r]
                B = T.reshape(128, 2, groups, 128)
                W = np.transpose(B[:, :, :, ::-1], (0, 2, 3, 1))
                out[c, eoff:eoff + n_el] = W.reshape(-1)
            else:
                out[c, eoff:eoff + n_el] = T.reshape(-1)
            doff += dspan
            eoff += n_el
    return out


def _gram_from_outputs(outs, swi=False) -> np.ndarray:
    """Sum the 16 diagonal 8x8 blocks of each core's [.,128,128] output."""
    G = np.zeros((N, N), dtype=np.float64)
    for O in outs:
        O = np.asarray(O, dtype=np.float64).reshape(-1, 128, 128)
        if swi:
            O = O[:, :, ::-1]   # moving operand streamed columns reversed
        O4 = O.reshape(-1, N, CC, N, CC)
        G += np.einsum('kicjc->ij', O4)
    return G


def _fw_solve(G: np.ndarray) -> np.ndarray:
    """Frank-Wolfe min-norm loop, replicating the reference fp32 semantics."""
    G = G.astype(np.float32)
    one = np.float32(1.0)
    sol = np.full(N, 1.0 / N, dtype=np.float32)
    for _ in range(MAX_ITER):
        gram_dot_sol = G @ sol
        t = int(np.argmin(gram_dot_sol))
        v1v1 = np.float32(np.dot(sol, gram_dot_sol))
        v1v2 = np.float32(np.dot(sol, G[:, t]))
        v2v2 = G[t, t]
        denom = np.float32(v1v1 + v2v2 - np.float32(2.0) * v1v2)
        with np.errstate(divide="ignore", invalid="ignore"):
            gamma = np.float32((v2v2 - v1v2) / denom)
        if v1v2 >= v2v2:
            gamma = np.float32(0.001)
        if v1v2 >= v1v1:
            gamma = np.float32(0.999)
        new_sol = (gamma * sol).astype(np.float32)
        new_sol[t] = np.float32(new_sol[t] + (one - gamma))
        change = np.float32(np.sum(np.abs(new_sol - sol)))
        sol = new_sol
        if change < np.float32(STOP_CRIT):
            break
    return sol


CONFIG = dict(in_dt="float8e4", warm=8, two_queues=True, swi=False)
SCHEDULE = default_tile_schedule(big=2048, head=(512, 1024), tail=(1024, 512))


def kernel(vecs) -> np.ndarray:
    from concourse.bass_utils import run_bass_kernel_spmd

    vecs = np.ascontiguousarray(np.asarray(vecs, dtype=np.float32))
    assert vecs.shape == (N, D)

    X = _pack(vecs, SCHEDULE, in_dt=CONFIG["in_dt"], swi=CONFIG["swi"])
    if "nc" not in _CACHE:
        _CACHE["nc"] = _build_nc(SCHEDULE, **CONFIG)
    nc = _CACHE["nc"]
    in_maps = [{"x": X[c]} for c in range(NCORES)]
    rr = run_bass_kernel_spmd(nc, in_maps, list(range(NCORES)))
    G = _gram_from_outputs((rr.results[c]["g"] for c in range(NCORES)),
                           swi=CONFIG["swi"])
    return _fw_solve(G)


# revision 4
# speedup vs baseline: 1.0596x; 1.0089x over previous
"""Trainium2 kernel for nn_MinNormSolverFW: min-norm Frank-Wolfe over 8 task
gradients of dimension 16777216.

Strategy (matches the sharding hint): shard the d axis across the 8 cores.
Each core computes a partial Gram matrix of its shard on the tensor engine;
the host sums the tiny partial Grams and runs the (negligible) Frank-Wolfe
loop, replicating the reference's fp32 semantics.

Device compute layout: the host pre-packs each core's shard so that every
128-column SBUF group holds 16 d-chunks x 8 vectors (columns m = i*16 + cc,
partitions = 128 d's per chunk, 2 contraction planes per column via
DoubleRow).  A single self-matmul (lhsT = rhs = group) accumulates all 16
chunk-level 8x8 outer products at full PE width into one [128,128] PSUM
region.  The host extracts the 16 diagonal 8x8 blocks of the output.

Input is quantized to fp8-e4m3 on the host (the final FW solution moves by
~4e-5 relative, far below tolerance), which quarters HBM traffic vs fp32.

v2 structure: every streaming tile gets its own dedicated SBUF buffer (the
full 16 MiB shard fits in SBUF), so DMAs never wait on buffer recycling and
stream back-to-back at fabric rate; the PE warm-up is only long enough to
cover the first tile's DMA flight; tiles ramp small -> large -> small so
the PE starts early and finishes right behind the last DMA.
"""
import numpy as np

N = 8                     # number of task vectors
D = 16777216              # vector dimension
NCORES = 8
CC = 16                   # d-chunks packed per matmul group (CC * N = 128)
DC = D // NCORES          # d per core
TOTAL_COLS = DC * N // 128 // 2  # double-row columns per core = 65536

MAX_ITER = 250
STOP_CRIT = 1e-06


def default_tile_schedule(big=4096, total=TOTAL_COLS,
                          head=(512, 1024, 2048), tail=(2048, 1024, 512)):
    head, tail = list(head), list(tail)
    mid_total = total - sum(head) - sum(tail)
    mid = [big] * (mid_total // big)
    rem = mid_total - big * len(mid)
    if rem:
        mid.append(rem)
    sched = head + mid + tail
    assert sum(sched) == total and all(c % 128 == 0 for c in sched)
    return sched


_CACHE = {}


def _np_dt(in_dt):
    if in_dt == "bfloat16":
        import ml_dtypes
        return ml_dtypes.bfloat16
    if in_dt == "float8e4":
        import ml_dtypes
        return ml_dtypes.float8_e4m3
    if in_dt == "float8e3":
        import ml_dtypes
        return ml_dtypes.float8_e3m4
    return np.float32


def _build_nc(schedule, in_dt="float8e4", warm=8, two_queues=True, swi=False):
    from concourse import bacc
    import concourse.mybir as mybir
    from concourse.tile import TileContext

    dt = getattr(mybir.dt, in_dt)
    total_cols = sum(schedule)
    n_mm = total_cols // 128
    perf_mode = (mybir.MatmulPerfMode.DoubleRowSwInterleave if swi
                 else mybir.MatmulPerfMode.DoubleRow)
    nc = bacc.Bacc("TRN2", debug=False)
    x = nc.dram_tensor("x", [128 * 2 * total_cols], dt, kind="ExternalInput")
    g_out = nc.dram_tensor("g", [1, 128, 128], mybir.dt.float32,
                           kind="ExternalOutput")
    with TileContext(nc) as tc:
        with tc.tile_pool(name="data", bufs=1) as pool, \
             tc.tile_pool(name="acc", bufs=1, space="PSUM") as ppool, \
             tc.tile_pool(name="warm", bufs=1) as wpool, \
             tc.tile_pool(name="res", bufs=1) as opool:
            acc = ppool.tile([128, 128], mybir.dt.float32)
            if warm:
                # Short PE pre-warm: overlaps the first tile's DMA flight and
                # starts the HAM activity window early.
                wt = wpool.tile([128, 128], mybir.dt.bfloat16)
                wacc = ppool.tile([128, 128], mybir.dt.float32, tag="wacc")
                nc.gpsimd.memset(wt[:], 0)
                for _ in range(warm):
                    nc.tensor.matmul(wacc[:], wt[:], wt[:],
                                     start=True, stop=True)
            k = 0
            off = 0
            for ti, cols in enumerate(schedule):
                # dedicated buffer per tile: no recycling dependencies
                if swi:
                    tile = pool.tile([128, 2 * cols], dt, tag=f"data{ti}",
                                     name=f"tile{ti}")
                else:
                    tile = pool.tile([128, 2, cols], dt, tag=f"data{ti}",
                                     name=f"tile{ti}")
                src = x[off:off + 256 * cols].rearrange(
                    "(p f) -> p f", p=128)
                if not swi:
                    src = src.rearrange("p (r c) -> p r c", r=2)
                eng = nc.scalar if (two_queues and ti % 2) else nc.sync
                eng.dma_start(out=tile[:], in_=src)
                off += 256 * cols
                for g in range(cols // 128):
                    if swi:
                        flat = tile[:, g * 256:(g + 1) * 256]
                        lhsT = flat
                        rhs = flat.rearrange("p (f two) -> p two f", two=2)
                    else:
                        lhsT = rhs = tile[:, :, g * 128:(g + 1) * 128]
                    nc.tensor.matmul(acc[:], lhsT, rhs,
                                     start=(k == 0),
                                     stop=(k == n_mm - 1),
                                     perf_mode=perf_mode)
                    k += 1
            res = opool.tile([128, 128], mybir.dt.float32, tag="res")
            nc.vector.tensor_copy(res[:], acc[:])
            nc.sync.dma_start(out=g_out[0], in_=res[:])
    assert k == n_mm
    nc.compile()
    return nc


def _pack(vecs: np.ndarray, schedule, in_dt="float8e4", swi=False,
          ncores=NCORES) -> np.ndarray:
    """[N, D] -> [ncores, 256*total_cols] flat packed device layout.

    Each 128-column matmul group holds 16 d-chunks x 8 vectors
    (column = i*16 + cc); a chunk spans 256 d's indexed by partition p
    and double-row plane r.  With swi, each group's 256 bytes per
    partition are stored interleaved-reversed: (A127,B127,...,A0,B0)
    where Ac/Bc are planes r=0/1 of logical column c, which is the
    layout DoubleRowSwInterleave's LDWEIGHTS reads contiguously.
    """
    np_dt = _np_dt(in_dt)
    dc = D // ncores
    q = vecs.astype(np_dt)
    out = np.empty((ncores, 256 * sum(schedule)), dtype=np_dt)
    for c in range(ncores):
        doff = 0
        eoff = 0
        Vc = q[:, c * dc:(c + 1) * dc]
        for cols in schedule:
            dspan = 256 * cols // N   # d per vector in this tile
            groups = cols // 128
            V = Vc[:, doff:doff + dspan].reshape(N, 128, 2, groups, CC)
            T = np.transpose(V, (1, 2, 3, 0, 4))     # [p, r, g, i, cc]
            n_el = 256 * cols
            if swi:
                # [p, r, g, col] -> [p, g, col(reversed), r]
                B = T.reshape(128, 2, groups, 128)
                W = np.transpose(B[:, :, :, ::-1], (0, 2, 3, 1))
                out[c, eoff:eoff + n_el] = W.reshape(-1)
            else:
                out[c, eoff:eoff + n_el] = T.reshape(-1)
            doff += dspan
            eoff += n_el
    return out


def _gram_from_outputs(outs, swi=False) -> np.ndarray:
    """Sum the 16 diagonal 8x8 blocks of each core's [.,128,128] output."""
    G = np.zeros((N, N), dtype=np.float64)
    for O in outs:
        O = np.asarray(O, dtype=np.float64).reshape(-1, 128, 128)
        if swi:
            O = O[:, :, ::-1]   # moving operand streamed columns reversed
        O4 = O.reshape(-1, N, CC, N, CC)
        G += np.einsum('kicjc->ij', O4)
    return G


def _fw_solve(G: np.ndarray) -> np.ndarray:
    """Frank-Wolfe min-norm loop, replicating the reference fp32 semantics."""
    G = G.astype(np.float32)
    one = np.float32(1.0)
    sol = np.full(N, 1.0 / N, dtype=np.float32)
    for _ in range(MAX_ITER):
        gram_dot_sol = G @ sol
        t = int(np.argmin(gram_dot_sol))
        v1v1 = np.float32(np.dot(sol, gram_dot_sol))
        v1v2 = np.float32(np.dot(sol, G[:, t]))
        v2v2 = G[t, t]
        denom = np.float32(v1v1 + v2v2 - np.float32(2.0) * v1v2)
        with np.errstate(divide="ignore", invalid="ignore"):
            gamma = np.float32((v2v2 - v1v2) / denom)
        if v1v2 >= v2v2:
            gamma = np.float32(0.001)
        if v1v2 >= v1v1:
            gamma = np.float32(0.999)
        new_sol = (gamma * sol).astype(np.float32)
        new_sol[t] = np.float32(new_sol[t] + (one - gamma))
        change = np.float32(np.sum(np.abs(new_sol - sol)))
        sol = new_sol
        if change < np.float32(STOP_CRIT):
            break
    return sol


CONFIG = dict(in_dt="float8e4", warm=8, two_queues=True, swi=True)
SCHEDULE = default_tile_schedule(big=2048, head=(512, 1024), tail=(1024, 512))


def kernel(vecs) -> np.ndarray:
    from concourse.bass_utils import run_bass_kernel_spmd

    vecs = np.ascontiguousarray(np.asarray(vecs, dtype=np.float32))
    assert vecs.shape == (N, D)

    X = _pack(vecs, SCHEDULE, in_dt=CONFIG["in_dt"], swi=CONFIG["swi"])
    if "nc" not in _CACHE:
        _CACHE["nc"] = _build_nc(SCHEDULE, **CONFIG)
    nc = _CACHE["nc"]
    in_maps = [{"x": X[c]} for c in range(NCORES)]
    rr = run_bass_kernel_spmd(nc, in_maps, list(range(NCORES)))
    G = _gram_from_outputs((rr.results[c]["g"] for c in range(NCORES)),
                           swi=CONFIG["swi"])
    return _fw_solve(G)


# revision 6
# speedup vs baseline: 1.0614x; 1.0017x over previous
"""Trainium2 kernel for nn_MinNormSolverFW: min-norm Frank-Wolfe over 8 task
gradients of dimension 16777216.

Strategy (matches the sharding hint): shard the d axis across the 8 cores.
Each core computes a partial Gram matrix of its shard on the tensor engine;
the host sums the tiny partial Grams and runs the (negligible) Frank-Wolfe
loop, replicating the reference's fp32 semantics.

Device compute layout: the host pre-packs each core's shard so that every
128-column SBUF group holds 16 d-chunks x 8 vectors (columns m = i*16 + cc,
partitions = 128 d's per chunk, 2 contraction planes per column via
DoubleRow).  A single self-matmul (lhsT = rhs = group) accumulates all 16
chunk-level 8x8 outer products at full PE width into one [128,128] PSUM
region.  The host extracts the 16 diagonal 8x8 blocks of the output.

Input is quantized to fp8-e4m3 on the host (the final FW solution moves by
~4e-5 relative, far below tolerance), which quarters HBM traffic vs fp32.

v2 structure: every streaming tile gets its own dedicated SBUF buffer (the
full 16 MiB shard fits in SBUF), so DMAs never wait on buffer recycling and
stream back-to-back at fabric rate; the PE warm-up is only long enough to
cover the first tile's DMA flight; tiles ramp small -> large -> small so
the PE starts early and finishes right behind the last DMA.
"""
import numpy as np

N = 8                     # number of task vectors
D = 16777216              # vector dimension
NCORES = 8
CC = 16                   # d-chunks packed per matmul group (CC * N = 128)
DC = D // NCORES          # d per core
TOTAL_COLS = DC * N // 128 // 2  # double-row columns per core = 65536

MAX_ITER = 250
STOP_CRIT = 1e-06


def default_tile_schedule(big=4096, total=TOTAL_COLS,
                          head=(512, 1024, 2048), tail=(2048, 1024, 512)):
    head, tail = list(head), list(tail)
    mid_total = total - sum(head) - sum(tail)
    mid = [big] * (mid_total // big)
    rem = mid_total - big * len(mid)
    if rem:
        mid.append(rem)
    sched = head + mid + tail
    assert sum(sched) == total and all(c % 128 == 0 for c in sched)
    return sched


_CACHE = {}


def _np_dt(in_dt):
    if in_dt == "bfloat16":
        import ml_dtypes
        return ml_dtypes.bfloat16
    if in_dt == "float8e4":
        import ml_dtypes
        return ml_dtypes.float8_e4m3
    if in_dt == "float8e3":
        import ml_dtypes
        return ml_dtypes.float8_e3m4
    return np.float32


def _build_nc(schedule, in_dt="float8e4", warm=8, two_queues=True, swi=False):
    from concourse import bacc
    import concourse.mybir as mybir
    from concourse.tile import TileContext

    dt = getattr(mybir.dt, in_dt)
    total_cols = sum(schedule)
    n_mm = total_cols // 128
    perf_mode = (mybir.MatmulPerfMode.DoubleRowSwInterleave if swi
                 else mybir.MatmulPerfMode.DoubleRow)
    nc = bacc.Bacc("TRN2", debug=False)
    x = nc.dram_tensor("x", [128 * 2 * total_cols], dt, kind="ExternalInput")
    g_out = nc.dram_tensor("g", [1, 128, 128], mybir.dt.float32,
                           kind="ExternalOutput")
    with TileContext(nc) as tc:
        with tc.tile_pool(name="data", bufs=1) as pool, \
             tc.tile_pool(name="acc", bufs=1, space="PSUM") as ppool, \
             tc.tile_pool(name="warm", bufs=1) as wpool, \
             tc.tile_pool(name="res", bufs=1) as opool:
            acc = ppool.tile([128, 128], mybir.dt.float32)
            if warm:
                # Short PE pre-warm: overlaps the first tile's DMA flight and
                # starts the HAM activity window early.
                wt = wpool.tile([128, 128], mybir.dt.bfloat16)
                wacc = ppool.tile([128, 128], mybir.dt.float32, tag="wacc")
                nc.gpsimd.memset(wt[:], 0)
                for _ in range(warm):
                    nc.tensor.matmul(wacc[:], wt[:], wt[:],
                                     start=True, stop=True)
            k = 0
            off = 0
            for ti, cols in enumerate(schedule):
                # dedicated buffer per tile: no recycling dependencies
                if swi:
                    tile = pool.tile([128, 2 * cols], dt, tag=f"data{ti}",
                                     name=f"tile{ti}")
                else:
                    tile = pool.tile([128, 2, cols], dt, tag=f"data{ti}",
                                     name=f"tile{ti}")
                src = x[off:off + 256 * cols].rearrange(
                    "(p f) -> p f", p=128)
                if not swi:
                    src = src.rearrange("p (r c) -> p r c", r=2)
                eng = nc.scalar if (two_queues and ti % 2 == 1) else nc.sync
                eng.dma_start(out=tile[:], in_=src)
                off += 256 * cols
                for g in range(cols // 128):
                    if swi:
                        flat = tile[:, g * 256:(g + 1) * 256]
                        lhsT = flat
                        rhs = flat.rearrange("p (f two) -> p two f", two=2)
                    else:
                        lhsT = rhs = tile[:, :, g * 128:(g + 1) * 128]
                    nc.tensor.matmul(acc[:], lhsT, rhs,
                                     start=(k == 0),
                                     stop=(k == n_mm - 1),
                                     perf_mode=perf_mode)
                    k += 1
            res = opool.tile([128, 128], mybir.dt.float32, tag="res")
            nc.vector.tensor_copy(res[:], acc[:])
            nc.scalar.dma_start(out=g_out[0], in_=res[:])
    assert k == n_mm
    nc.compile()
    return nc


def _pack(vecs: np.ndarray, schedule, in_dt="float8e4", swi=False,
          ncores=NCORES) -> np.ndarray:
    """[N, D] -> [ncores, 256*total_cols] flat packed device layout.

    Each 128-column matmul group holds 16 d-chunks x 8 vectors
    (column = i*16 + cc); a chunk spans 256 d's indexed by partition p
    and double-row plane r.  With swi, each group's 256 bytes per
    partition are stored interleaved-reversed: (A127,B127,...,A0,B0)
    where Ac/Bc are planes r=0/1 of logical column c, which is the
    layout DoubleRowSwInterleave's LDWEIGHTS reads contiguously.
    """
    np_dt = _np_dt(in_dt)
    dc = D // ncores
    q = vecs.astype(np_dt)
    out = np.empty((ncores, 256 * sum(schedule)), dtype=np_dt)
    for c in range(ncores):
        doff = 0
        eoff = 0
        Vc = q[:, c * dc:(c + 1) * dc]
        for cols in schedule:
            dspan = 256 * cols // N   # d per vector in this tile
            groups = cols // 128
            V = Vc[:, doff:doff + dspan].reshape(N, 128, 2, groups, CC)
            T = np.transpose(V, (1, 2, 3, 0, 4))     # [p, r, g, i, cc]
            n_el = 256 * cols
            if swi:
                # [p, r, g, col] -> [p, g, col(reversed), r]
                B = T.reshape(128, 2, groups, 128)
                W = np.transpose(B[:, :, :, ::-1], (0, 2, 3, 1))
                out[c, eoff:eoff + n_el] = W.reshape(-1)
            else:
                out[c, eoff:eoff + n_el] = T.reshape(-1)
            doff += dspan
            eoff += n_el
    return out


def _gram_from_outputs(outs, swi=False) -> np.ndarray:
    """Sum the 16 diagonal 8x8 blocks of each core's [.,128,128] output."""
    G = np.zeros((N, N), dtype=np.float64)
    for O in outs:
        O = np.asarray(O, dtype=np.float64).reshape(-1, 128, 128)
        if swi:
            O = O[:, :, ::-1]   # moving operand streamed columns reversed
        O4 = O.reshape(-1, N, CC, N, CC)
        G += np.einsum('kicjc->ij', O4)
    return G


def _fw_solve(G: np.ndarray) -> np.ndarray:
    """Frank-Wolfe min-norm loop, replicating the reference fp32 semantics."""
    G = G.astype(np.float32)
    one = np.float32(1.0)
    sol = np.full(N, 1.0 / N, dtype=np.float32)
    for _ in range(MAX_ITER):
        gram_dot_sol = G @ sol
        t = int(np.argmin(gram_dot_sol))
        v1v1 = np.float32(np.dot(sol, gram_dot_sol))
        v1v2 = np.float32(np.dot(sol, G[:, t]))
        v2v2 = G[t, t]
        denom = np.float32(v1v1 + v2v2 - np.float32(2.0) * v1v2)
        with np.errstate(divide="ignore", invalid="ignore"):
            gamma = np.float32((v2v2 - v1v2) / denom)
        if v1v2 >= v2v2:
            gamma = np.float32(0.001)
        if v1v2 >= v1v1:
            gamma = np.float32(0.999)
        new_sol = (gamma * sol).astype(np.float32)
        new_sol[t] = np.float32(new_sol[t] + (one - gamma))
        change = np.float32(np.sum(np.abs(new_sol - sol)))
        sol = new_sol
        if change < np.float32(STOP_CRIT):
            break
    return sol


CONFIG = dict(in_dt="float8e4", warm=8, two_queues=False, swi=True)
SCHEDULE = default_tile_schedule(big=2048, head=(512, 1024), tail=(1024, 512))


def kernel(vecs) -> np.ndarray:
    from concourse.bass_utils import run_bass_kernel_spmd

    vecs = np.ascontiguousarray(np.asarray(vecs, dtype=np.float32))
    assert vecs.shape == (N, D)

    X = _pack(vecs, SCHEDULE, in_dt=CONFIG["in_dt"], swi=CONFIG["swi"])
    if "nc" not in _CACHE:
        _CACHE["nc"] = _build_nc(SCHEDULE, **CONFIG)
    nc = _CACHE["nc"]
    in_maps = [{"x": X[c]} for c in range(NCORES)]
    rr = run_bass_kernel_spmd(nc, in_maps, list(range(NCORES)))
    G = _gram_from_outputs((rr.results[c]["g"] for c in range(NCORES)),
                           swi=CONFIG["swi"])
    return _fw_solve(G)
